# revision 1
# baseline (speedup 1.0000x reference)
"""AdditiveAttentionPooling on 8 TRN2 NeuronCores (Bass/Tile).

Data-parallel over batch: B=32 rows -> 4 rows per core, no collectives.
Single streaming pass over x (bf16): per DMA tile, one VectorE tensor_mul
computes the x*w products (2x packed bf16 mode) and one pairwise fold
halves them; the per-slice free-dim reduction to scores is split between
ScalarE (activation Copy+accum, with b+mask_bias folded in for free via
the activation's affine stage: bias = mebias/512 added to each of the 512
accumulated elements) and VectorE (tensor_reduce + a [128,1] bias add) to
balance the engines. One Exp per tile produces the softmax numerator
weights p (shift-invariant softmax, O(1) scores; masked positions get
-100 so exp underflows to 0 exactly like the reference's -1e9 path).
TensorE accumulates p^T @ x into PSUM plus a ones-matmul denominator;
normalize and store per row.
"""

import sys

sys.path.insert(0, "/opt/trn_rl_repo")

import numpy as np

import concourse.bass as bass
import concourse.tile as tile
from concourse import mybir
from concourse.bass_utils import run_bass_kernel_spmd
from concourse.vector_clock import ScopedClock

N_CORES = 8
B, T, D = 32, 2048, 1024
ROWS_PER_CORE = B // N_CORES          # 4
TOK_PER_CORE = ROWS_PER_CORE * T      # 8192
P = 128                               # SBUF partitions
JMAX = 4                              # max tokens per partition per DMA
SLICES_PER_ROW = T // P               # 16
MASK_NEG = -100.0                     # exp(-100) ~ 4e-44 ~ 0 in f32

F32 = mybir.dt.float32
MODE = "bf16"


def core_schedule(r):
    """Per-row DMA tiling: (token offset within row, tokens-per-partition).
    Row 0 ramps up from small tiles so compute begins sooner after launch;
    later rows use big 2 MiB DMAs for efficiency."""
    if r == 0:
        return [(0, 2), (256, 2), (512, 4), (1024, 4), (1536, 4)]
    if r == ROWS_PER_CORE - 1:
        # ramp DOWN at the end: the final tile's serial dependency chain
        # (mul->fold->reduce->exp->matmuls->normalize) sets the tail length
        return [(0, 4), (512, 4), (1024, 4), (1536, 2), (1792, 1), (1920, 1)]
    return [(512 * h, 4) for h in range(4)]


def fold_tile(r, tno):
    """Tiles whose products get the pairwise DVE fold before reduction.
    Skipped on the first tiles (ScalarE is idle during pipeline fill and can
    absorb the longer unfolded reduce) and on two mid-stream tiles (shifts
    work from the busier VectorE to ScalarE)."""
    if r == 0 and tno < 2:
        return False
    return True


# slices c % 8 == 5 reduce on VectorE, the rest on ScalarE — balances the
# two engines given VectorE also does all the multiplies and folds
def reduce_engine(r, c):
    if r == ROWS_PER_CORE - 1 and c >= 14:
        # keep the kernel's final dependency chain on VectorE (its stream has
        # drained by then; ScalarE may still have an accum backlog)
        return "dve"
    return "dve" if c % 8 == 5 else "act"


# ---------------------------------------------------------------------------
# walrus-compat patches: the walrus build in this container rejects any
# instruction carrying more than one sync-wait ("Too many sync wait
# commands"), while Tile freely attaches one wait per producer. Split the
# extras onto NoOp instructions committed just before on the same engine
# (sequential on one engine => identical semantics).
# ---------------------------------------------------------------------------

_orig_commit = tile.TileContext._commit_instruction


def _commit_split_waits(self, inst, lazy_reg_writes=True):
    si = getattr(inst, "sync_info", None)
    if si is not None and si.on_wait is not None and len(si.on_wait) > 1:
        waits = list(si.on_wait)
        si.on_wait = waits[-1:]
        nop = mybir.InstNoOp(
            name=self.nc.get_next_instruction_name(),
            engine=inst.engine,
            bass_nofuse=True,
            sync_info=mybir.SyncInfo(on_wait=waits[:-1], on_update=[]),
        )
        _commit_split_waits(self, nop, lazy_reg_writes)
    return _orig_commit(self, inst, lazy_reg_writes)


tile.TileContext._commit_instruction = _commit_split_waits


def _drain_and_barrier_split_waits(self, tick_clock, wait_clock):
    """Same single-wait constraint for the kernel-tail drain: spread its
    per-DMA-lane waits over a chain of drain instructions on SyncE."""
    nc = self.nc
    drain_inst = nc.sync.drain()
    wait_clock.add_sem_waits(
        drain_inst.ins, ScopedClock({None: tick_clock.global_clock})
    )
    waits = list(drain_inst.ins.sync_info.on_wait)
    if len(waits) > 1:
        drain_inst.ins.sync_info.on_wait = [waits[0]]
        for w in waits[1:]:
            extra = nc.sync.drain()
            extra.ins.sync_info = mybir.SyncInfo(on_wait=[w], on_update=[])
    nc.all_engine_barrier()
    popped = nc._tile_sem_poison_stack.pop()
    assert popped is self._sem_poison
    nc.clear_and_free_semaphores(list(self.sems.allocated().values()))
    nc.all_engine_barrier()


tile.TileContext._drain_and_barrier = _drain_and_barrier_split_waits

# ---------------------------------------------------------------------------


def build_graph(mode=None):
    mode = mode or MODE
    DT = mybir.dt.bfloat16 if mode == "bf16" else mybir.dt.float32r

    nc = bass.Bass()
    x = nc.declare_dram_parameter("x", [TOK_PER_CORE, D], DT, isOutput=False)
    wrep = nc.declare_dram_parameter("wrep", [P, D], DT, isOutput=False)
    # mebias[p, r*16 + c] = b + (0 if mask else MASK_NEG); mebias_s = mebias/512
    NCOL = ROWS_PER_CORE * SLICES_PER_ROW
    mebias = nc.declare_dram_parameter("mebias", [P, NCOL], F32, isOutput=False)
    mebias_s = nc.declare_dram_parameter("mebias_s", [P, NCOL], F32, isOutput=False)
    out = nc.declare_dram_parameter("out", [ROWS_PER_CORE, D], F32, isOutput=True)

    with tile.TileContext(nc) as tc:
        with (
            tc.tile_pool(name="xpool", bufs=8) as xpool,
            tc.tile_pool(name="singles", bufs=1) as singles,
            tc.tile_pool(name="prodp", bufs=4) as prodp,
            tc.tile_pool(name="small", bufs=6) as small,
            tc.tile_pool(name="epi", bufs=2) as epi,
            tc.tile_pool(name="psum", bufs=2, space="PSUM") as psum_pool,
        ):
            wrep_t = singles.tile([P, D], DT)
            nc.scalar.dma_start(out=wrep_t, in_=wrep[:, :])
            mebias_t = singles.tile([P, NCOL], F32)
            nc.gpsimd.dma_start(out=mebias_t, in_=mebias[:, :])
            mebias_s_t = singles.tile([P, NCOL], F32)
            nc.gpsimd.dma_start(out=mebias_s_t, in_=mebias_s[:, :])
            ones_t = singles.tile([P, 1], DT)
            nc.vector.memset(ones_t, 1.0)
            # stride-0 broadcast target for ScalarE-accum elementwise output
            scr_t = singles.tile([P, 1], DT)

            def emit_epilogue(er, eps_n, eps_den):
                # normalize row er and store
                rden_t = epi.tile([1, 1], F32, tag="rden")
                nc.vector.reciprocal(rden_t, eps_den[:, 0:1])
                o_t = epi.tile([1, D], F32, tag="o")
                if er == ROWS_PER_CORE - 1:
                    # VectorE is idle by now; same-engine hop after the recip
                    # shortens the final serial chain
                    nc.vector.tensor_scalar_mul(out=o_t, in0=eps_n, scalar1=rden_t)
                else:
                    nc.scalar.activation(
                        out=o_t,
                        in_=eps_n,
                        func=mybir.ActivationFunctionType.Identity,
                        bias=0.0,
                        scale=rden_t,
                    )
                nc.sync.dma_start(out=out[er : er + 1, :], in_=o_t)

            pending = None
            for r in range(ROWS_PER_CORE):
                ps_n = psum_pool.tile([1, 1024], F32, tag="ps_n")
                ps_den = psum_pool.tile([1, 2], F32, tag="ps_den")
                for tno, (t0r, jc) in enumerate(core_schedule(r)):
                    if tno == 1 and pending is not None:
                        emit_epilogue(*pending)
                        pending = None
                    t0 = r * T + t0r
                    xt = xpool.tile([P, JMAX * D], DT, tag="xt")
                    xtv = xt[:, : jc * D]
                    nc.sync.dma_start(
                        out=xtv,
                        in_=x[t0 : t0 + jc * P, :].rearrange(
                            "(p j) d -> p (j d)", p=P
                        ),
                    )
                    # one batched multiply for the whole DMA tile (jc slices),
                    # then one pairwise fold so each reduce only reads D/2
                    prod_t = prodp.tile([P, JMAX * D], DT, tag="prod")
                    xt3 = xtv.rearrange("p (j d) -> p j d", j=jc)
                    w3 = bass.AP(
                        tensor=wrep_t.tensor,
                        offset=wrep_t.offset,
                        ap=[wrep_t.ap[0], [0, jc], [1, D]],
                    )
                    prod3 = prod_t[:, : jc * D].rearrange("p (j d) -> p j d", j=jc)
                    nc.vector.tensor_mul(prod3, xt3, w3)
                    folded = fold_tile(r, tno)
                    if folded:
                        fold_t = prodp.tile([P, JMAX * (D // 2)], DT, tag="fold")
                        fold3 = fold_t[:, : jc * (D // 2)].rearrange(
                            "p (j d) -> p j d", j=jc
                        )
                        nc.vector.tensor_add(
                            fold3, prod3[:, :, 0 : D // 2], prod3[:, :, D // 2 : D]
                        )
                        red_src, red_w = fold_t, D // 2
                    else:
                        red_src, red_w = prod_t, D
                    c0 = t0r // P
                    s4 = small.tile([P, JMAX], F32, tag="s4")
                    for j in range(jc):
                        c = c0 + j
                        col = r * SLICES_PER_ROW + c
                        fslice = red_src[:, j * red_w : (j + 1) * red_w]
                        if reduce_engine(r, c) == "act":
                            # the affine stage adds bias to each of the D/2
                            # accumulated elements, so mebias/512 sums to the
                            # exact mask bias (-100/512 is binary-exact)
                            nc.scalar.activation(
                                out=scr_t.broadcast_to((P, red_w)),
                                in_=fslice,
                                func=mybir.ActivationFunctionType.Identity,
                                bias=mebias_s_t[:, col : col + 1],
                                scale=1.0,
                                accum_out=s4[:, j : j + 1],
                            )
                        else:
                            nc.vector.tensor_reduce(
                                out=s4[:, j : j + 1],
                                in_=fslice,
                                op=mybir.AluOpType.add,
                                axis=mybir.AxisListType.X,
                            )
                            nc.vector.tensor_add(
                                s4[:, j : j + 1],
                                s4[:, j : j + 1],
                                mebias_t[:, col : col + 1],
                            )
                    # p = exp(s) for the tile's jc slices at once
                    p4 = small.tile([P, JMAX], DT, tag="p4")
                    nc.scalar.activation(
                        out=p4[:, :jc],
                        in_=s4[:, :jc],
                        func=mybir.ActivationFunctionType.Exp,
                    )
                    for j in range(jc):
                        c = c0 + j
                        xs = xtv[:, j * D : (j + 1) * D]
                        first = c == 0
                        last = c == SLICES_PER_ROW - 1
                        nc.tensor.matmul(
                            ps_n[:, 0:512],
                            lhsT=p4[:, j : j + 1],
                            rhs=xs[:, 0:512],
                            start=first,
                            stop=last,
                        )
                        nc.tensor.matmul(
                            ps_n[:, 512:1024],
                            lhsT=p4[:, j : j + 1],
                            rhs=xs[:, 512:1024],
                            start=first,
                            stop=last,
                        )
                        nc.tensor.matmul(
                            ps_den[:, 0:1],
                            lhsT=p4[:, j : j + 1],
                            rhs=ones_t,
                            start=first,
                            stop=last,
                        )
                pending = (r, ps_n, ps_den)
            emit_epilogue(*pending)
    return nc


def make_in_maps(x, mask, w, b, mode=None):
    mode = mode or MODE
    if mode == "bf16":
        import ml_dtypes

        np_dt = ml_dtypes.bfloat16
    else:
        np_dt = np.float32
    wrep = np.ascontiguousarray(
        np.broadcast_to(np.asarray(w).ravel()[None, :], (P, D))
    ).astype(np_dt)
    bval = float(np.asarray(b).ravel()[0])
    in_maps = []
    for core in range(N_CORES):
        rows = slice(core * ROWS_PER_CORE, (core + 1) * ROWS_PER_CORE)
        xc = np.ascontiguousarray(
            np.asarray(x[rows]).reshape(TOK_PER_CORE, D).astype(np_dt)
        )
        m = np.asarray(mask[rows], bool)
        mb = np.empty((P, ROWS_PER_CORE * SLICES_PER_ROW), np.float32)
        parts = np.arange(P)
        for r in range(ROWS_PER_CORE):
            for t0r, jc in core_schedule(r):
                for j in range(jc):
                    c = t0r // P + j
                    toks = t0r + jc * parts + j
                    mb[:, r * SLICES_PER_ROW + c] = np.where(
                        m[r, toks], bval, bval + MASK_NEG
                    )
        mb = np.ascontiguousarray(mb)
        div = np.full(ROWS_PER_CORE * SLICES_PER_ROW, D // 2, np.float32)
        for r in range(ROWS_PER_CORE):
            for tno, (t0r, jc) in enumerate(core_schedule(r)):
                if not fold_tile(r, tno):
                    c0 = t0r // P
                    div[r * SLICES_PER_ROW + c0 : r * SLICES_PER_ROW + c0 + jc] = D
        mbs = np.ascontiguousarray(mb / div[None, :])
        in_maps.append({"x": xc, "wrep": wrep, "mebias": mb, "mebias_s": mbs})
    return in_maps


def run(x, mask, w, b, trace=False, mode=None):
    nc = build_graph(mode)
    in_maps = make_in_maps(x, mask, w, b, mode)
    res = run_bass_kernel_spmd(nc, in_maps, core_ids=list(range(N_CORES)), trace=trace)
    out = np.concatenate([res.results[i]["out"] for i in range(N_CORES)], axis=0)
    return out, res


def kernel(x, mask, w, b):
    out, _ = run(x, mask, w, b, trace=False)
    return out

